# revision 1
# baseline (speedup 1.0000x reference)
"""Causal self-attention Trainium2 kernel.

B=1024, S=77, E=1024, H=16, D=64. Data-parallel over batch across 8 cores
(128 batches/core). Inside each core everything runs in bf16 on the PE with
fp32 PSUM accumulation; softmax runs in fp32 on ACT/DVE.

Layout scheme (per core, T = 128*77 = 9856 tokens):
  xT      [E=1024, T]    bf16 (host pre-transposed)
  Q^T,K^T [2048 f, T]    bf16 = W_qk^T @ x^T   (f on partitions, chunk 128)
  V       [77, 1024] x6  bf16 = x_b @ W_v      (per batch, tokens on partitions)
  scores  [77 q, 77 k]   head-pair row-packed matmuls (K=64, rows 0/64)
  att     softmax in fp32 (mask add on DVE, exp on ACT, segmented sum on DVE)
  att^T   PE transpose -> bf16
  O^T     [128 f, 77] head-pair col-packed matmuls (tile_position=(0,64))
  y^T     [E, T] fp32 = W_p^T @ O^T, host transposes back
"""

import sys

sys.path.insert(0, "/opt/trn_rl_repo")

import numpy as np
import ml_dtypes

import concourse.bass as bass
import concourse.mybir as mybir
import concourse.tile as tile
from concourse import bacc
from concourse.bass_utils import run_bass_kernel_spmd

F32 = mybir.dt.float32
BF16 = mybir.dt.bfloat16
AX = mybir.AxisListType
AF = mybir.ActivationFunctionType

N_CORES = 8
B, S, E = 1024, 77, 1024
H, D = 16, 64
BC = B // N_CORES          # batches per core = 128
T = BC * S                 # tokens per core = 9856
SCALE = 1.0 / float(np.sqrt(D))
NEG = -1.0e30

# block structure: 21 blocks of 6 batches + 1 block of 2
BLOCKS = [(i * 6, 6) for i in range(21)] + [(126, 2)]


def _emit_block(nc, tc, P, b0, G):
    Tb = G * S                       # tokens this block
    t0 = b0 * S
    xt = []
    for e in range(8):
        xtile = P["x"].tile([128, Tb], BF16, tag=f"xt{e}")
        nc.sync.dma_start(xtile[:], P["xT"][128 * e:128 * (e + 1), t0:t0 + Tb])
        xt.append(xtile)

    # ---- Q^T / K^T GEMM: 16 f-chunks of 128, contraction over 8 e-chunks
    qk = []
    for c in range(16):
        ps = P["gps"].tile([128, 512], F32, tag="g")
        for e in range(8):
            nc.tensor.matmul(
                ps[:, :Tb],
                P["wqk"][e][:, 128 * c:128 * (c + 1)],
                xt[e][:],
                start=(e == 0), stop=(e == 7),
            )
        o = P["qk"].tile([128, Tb], BF16, tag=f"qk{c}")
        # Identity(ps*scale + bias): SCALE folded into Q here (bias pre-scaled on host)
        nc.scalar.activation(
            o[:], ps[:, :Tb], AF.Identity,
            bias=P["bqk"][:, c:c + 1], scale=(SCALE if c < 8 else 1.0),
        )
        qk.append(o)

    # ---- V GEMM per batch: out [77 tokens, 1024 f]
    vb = []
    for g in range(G):
        v = P["v"].tile([77, 1024], BF16, tag=f"v{g}")
        for fc in range(2):
            ps = P["gps"].tile([128, 512], F32, tag="g")
            for e in range(8):
                nc.tensor.matmul(
                    ps[:77, :],
                    xt[e][:, S * g:S * (g + 1)],
                    P["wv"][e][:, 512 * fc:512 * (fc + 1)],
                    start=(e == 0), stop=(e == 7),
                )
            nc.vector.tensor_add(
                v[:, 512 * fc:512 * (fc + 1)],
                ps[:77, :],
                P["bv"][:77, 512 * fc:512 * (fc + 1)],
            )
        vb.append(v)

    # ---- attention. HW constraint: matmuls within one PSUM bank must not
    # alternate row groups -> even heads (rows 0:64) and odd heads (rows
    # 64:128) accumulate in separate score banks, emitted interleaved so PE
    # overlaps LDWEIGHTS across row groups.
    def _softmax_transpose(sc, grp):
        n = len(grp)
        W = n * S
        t3 = P["sm"].tile([77, 462], F32, tag="tsb", name="tsb")
        mask_bc = P["mask"][:77, :].unsqueeze(1).broadcast_to([77, n, S])
        nc.vector.tensor_add(
            t3[:, :W].rearrange("p (n k) -> p n k", k=S),
            sc[:, :W].rearrange("p (n k) -> p n k", k=S),
            mask_bc,
        )
        eb = P["sm"].tile([77, 462], F32, tag="esb", name="esb")
        nc.scalar.activation(eb[:, :W], t3[:, :W], AF.Exp)
        z = P["z"].tile([77, 8], F32, tag="z", name="z")
        nc.vector.reduce_sum(
            z[:, :n].unsqueeze(2),
            eb[:, :W].rearrange("p (n k) -> p n k", k=S),
            axis=AX.X,
        )
        zr = P["z"].tile([77, 8], F32, tag="zr", name="zr")
        nc.vector.reciprocal(zr[:, :n], z[:, :n])
        att = P["sm"].tile([77, 462], F32, tag="attsb", name="attsb")
        zr_bc = zr[:, :n].unsqueeze(2).broadcast_to([77, n, S])
        nc.vector.tensor_mul(
            att[:, :W].rearrange("p (n k) -> p n k", k=S),
            eb[:, :W].rearrange("p (n k) -> p n k", k=S),
            zr_bc,
        )
        tp_full = P["atps"].tile([128, 512], F32, tag="at", name="at")
        tp = tp_full[:77, :]
        for i in range(n):
            nc.tensor.transpose(
                tp[:, S * i:S * (i + 1)], att[:, S * i:S * (i + 1)], P["ident"][:]
            )
        aT = P["attT"].tile([77, 462], BF16, tag="attT", name="attT")
        nc.scalar.activation(aT[:, :W], tp[:, :W], AF.Copy)
        return aT

    evens = [(g, h) for g in range(G) for h in range(0, H, 2)]
    odds = [(g, h) for g in range(G) for h in range(1, H, 2)]
    egroups = [evens[i:i + 6] for i in range(0, len(evens), 6)]
    ogroups = [odds[i:i + 6] for i in range(0, len(odds), 6)]
    attT = []
    pair_loc = {}
    for eg, og in zip(egroups, ogroups):
        scA_f = P["scps"].tile([128, 512], F32, tag="sc", name="sc")
        scB_f = P["scps"].tile([128, 512], F32, tag="sc", name="sc")
        scA, scB = scA_f[:77, :], scB_f[:77, :]
        for i in range(len(eg)):
            gA, hA = eg[i]
            gB, hB = og[i]
            cA, cB = hA // 2, hB // 2
            nc.tensor.matmul(
                scA[:, S * i:S * (i + 1)],
                qk[cA][0:64, S * gA:S * (gA + 1)],
                qk[8 + cA][0:64, S * gA:S * (gA + 1)],
                start=True, stop=True,
            )
            nc.tensor.matmul(
                scB[:, S * i:S * (i + 1)],
                qk[cB][64:128, S * gB:S * (gB + 1)],
                qk[8 + cB][64:128, S * gB:S * (gB + 1)],
                start=True, stop=True,
            )
        aT_A = _softmax_transpose(scA, eg)
        for i, pr in enumerate(eg):
            pair_loc[pr] = (len(attT), i)
        attT.append(aT_A)
        aT_B = _softmax_transpose(scB, og)
        for i, pr in enumerate(og):
            pair_loc[pr] = (len(attT), i)
        attT.append(aT_B)

    # ---- O^T: head-pair col-packed matmuls, PSUM bank per pair-index j
    ot = []
    for j in range(8):
        ps2 = P["m2ps"].tile([128, 512], F32, tag="m2", name="m2")
        for g in range(G):
            giE, slE = pair_loc[(g, 2 * j)]
            giO, slO = pair_loc[(g, 2 * j + 1)]
            nc.tensor.matmul(
                ps2[0:64, S * g:S * (g + 1)],
                vb[g][:, 64 * (2 * j):64 * (2 * j) + 64],
                attT[giE][:, S * slE:S * (slE + 1)],
                start=True, stop=True,
            )
            nc.tensor.matmul(
                ps2[64:128, S * g:S * (g + 1)],
                vb[g][:, 64 * (2 * j + 1):64 * (2 * j + 1) + 64],
                attT[giO][:, S * slO:S * (slO + 1)],
                start=True, stop=True,
                tile_position=(0, 64),
            )
        o = P["ot"].tile([128, Tb], BF16, tag=f"ot{j}")
        nc.scalar.activation(o[:], ps2[:, :Tb], AF.Copy)
        ot.append(o)

    # ---- projection: y^T[e-chunk, t] = sum_j Wp[j-chunk]^T @ O^T[j]
    for ec in range(8):
        ps = P["gps"].tile([128, 512], F32, tag="g")
        for j in range(8):
            nc.tensor.matmul(
                ps[:, :Tb],
                P["wp"][j][:, 128 * ec:128 * (ec + 1)],
                ot[j][:],
                start=(j == 0), stop=(j == 7),
            )
        y = P["y"].tile([128, Tb], F32, tag="y")
        nc.scalar.activation(
            y[:], ps[:, :Tb], AF.Identity, bias=P["bp"][:, ec:ec + 1]
        )
        nc.sync.dma_start(P["yT"][128 * ec:128 * (ec + 1), t0:t0 + Tb], y[:])


def build(blocks=None):
    if blocks is None:
        blocks = BLOCKS
    nc = bacc.Bacc(None)
    xT = nc.dram_tensor("xT", [E, T], BF16, kind="ExternalInput")
    wqk_d = nc.dram_tensor("wqk", [E, 2048], BF16, kind="ExternalInput")
    wv_d = nc.dram_tensor("wv", [E, 1024], BF16, kind="ExternalInput")
    wp_d = nc.dram_tensor("wp", [1024, 1024], BF16, kind="ExternalInput")
    bqk_d = nc.dram_tensor("bqk", [128, 16], F32, kind="ExternalInput")
    bv_d = nc.dram_tensor("bv", [128, 1024], F32, kind="ExternalInput")
    bp_d = nc.dram_tensor("bp", [128, 8], F32, kind="ExternalInput")
    mask_d = nc.dram_tensor("mask", [77, 77], F32, kind="ExternalInput")
    id_d = nc.dram_tensor("ident", [77, 77], F32, kind="ExternalInput")
    yT = nc.dram_tensor("yT", [E, T], F32, kind="ExternalOutput")

    with tile.TileContext(nc) as tc:
        with (
            tc.tile_pool(name="w", bufs=1) as wpool,
            tc.tile_pool(name="x", bufs=2) as xpool,
            tc.tile_pool(name="qk", bufs=1) as qkpool,
            tc.tile_pool(name="v", bufs=1) as vpool,
            tc.tile_pool(name="sm", bufs=3) as smpool,
            tc.tile_pool(name="z", bufs=3) as zpool,
            tc.tile_pool(name="attT", bufs=18) as attTpool,
            tc.tile_pool(name="ot", bufs=2) as otpool,
            tc.tile_pool(name="y", bufs=3) as ypool,
            tc.tile_pool(name="gps", bufs=2, space="PSUM") as gpspool,
            tc.tile_pool(name="scps", bufs=3, space="PSUM") as scpool,
            tc.tile_pool(name="atps", bufs=1, space="PSUM") as atpool,
            tc.tile_pool(name="m2ps", bufs=2, space="PSUM") as m2pool,
        ):
            P = {}
            P["wqk"] = []
            P["wv"] = []
            P["wp"] = []
            for e in range(8):
                w1 = wpool.tile([128, 2048], BF16, tag=f"wqk{e}", name=f"wqk{e}")
                nc.sync.dma_start(w1[:], wqk_d[128 * e:128 * (e + 1), :])
                P["wqk"].append(w1)
                w2 = wpool.tile([128, 1024], BF16, tag=f"wv{e}", name=f"wv{e}")
                nc.sync.dma_start(w2[:], wv_d[128 * e:128 * (e + 1), :])
                P["wv"].append(w2)
                w3 = wpool.tile([128, 1024], BF16, tag=f"wp{e}", name=f"wp{e}")
                nc.sync.dma_start(w3[:], wp_d[128 * e:128 * (e + 1), :])
                P["wp"].append(w3)
            P["bqk"] = wpool.tile([128, 16], F32, tag="bqk", name="bqk")
            nc.sync.dma_start(P["bqk"][:], bqk_d[:])
            P["bv"] = wpool.tile([128, 1024], F32, tag="bv", name="bv")
            nc.sync.dma_start(P["bv"][:], bv_d[:])
            P["bp"] = wpool.tile([128, 8], F32, tag="bp", name="bp")
            nc.sync.dma_start(P["bp"][:], bp_d[:])
            P["mask"] = wpool.tile([77, 77], F32, tag="mask", name="mask")
            nc.sync.dma_start(P["mask"][:], mask_d[:])
            P["ident"] = wpool.tile([77, 77], F32, tag="ident", name="ident")
            nc.sync.dma_start(P["ident"][:], id_d[:])
            P["xT"] = xT
            P["yT"] = yT
            P["x"] = xpool
            P["qk"] = qkpool
            P["v"] = vpool
            P["sm"] = smpool
            P["z"] = zpool
            P["attT"] = attTpool
            P["ot"] = otpool
            P["y"] = ypool
            P["gps"] = gpspool
            P["scps"] = scpool
            P["atps"] = atpool
            P["m2ps"] = m2pool

            for b0, G in blocks:
                _emit_block(nc, tc, P, b0, G)

    nc.finalize()
    return nc


_CACHE = {}


def _get_nc():
    if "nc" not in _CACHE:
        _CACHE["nc"] = build()
    return _CACHE["nc"]


def make_inputs(x, W_attn, b_attn, W_proj, b_proj):
    """Host-side prep: shard + transpose + cast. Returns per-core input maps."""
    x = np.asarray(x, dtype=np.float32)
    W_attn = np.asarray(W_attn, dtype=np.float32)
    b_attn = np.asarray(b_attn, dtype=np.float32)
    W_proj = np.asarray(W_proj, dtype=np.float32)
    b_proj = np.asarray(b_proj, dtype=np.float32)

    wqk = W_attn[:, :2048].astype(ml_dtypes.bfloat16)
    wv = W_attn[:, 2048:].astype(ml_dtypes.bfloat16)
    wp = W_proj.astype(ml_dtypes.bfloat16)
    # bias chunks [128, 16]: col c = b_attn[128c:128c+128]; Q part pre-scaled
    bq = b_attn[:2048].copy()
    bq[:1024] *= SCALE
    bqk = np.stack([bq[128 * c:128 * (c + 1)] for c in range(16)], axis=1).astype(np.float32)
    bv = np.broadcast_to(b_attn[2048:], (128, 1024)).copy().astype(np.float32)
    bp = np.stack([b_proj[128 * c:128 * (c + 1)] for c in range(8)], axis=1).astype(np.float32)
    mask = np.where(
        np.tril(np.ones((77, 77), dtype=bool)), 0.0, NEG
    ).astype(np.float32)
    ident = np.eye(77, dtype=np.float32)

    maps = []
    for cid in range(N_CORES):
        xs = x[BC * cid:BC * (cid + 1)].reshape(T, E)
        xTc = np.ascontiguousarray(xs.T).astype(ml_dtypes.bfloat16)
        maps.append({
            "xT": xTc, "wqk": wqk, "wv": wv, "wp": wp,
            "bqk": bqk, "bv": bv, "bp": bp, "mask": mask, "ident": ident,
        })
    return maps


def assemble_output(results):
    y = np.empty((B, S, E), dtype=np.float32)
    for cid in range(N_CORES):
        yTc = results[cid]["yT"]  # [E, T]
        y[BC * cid:BC * (cid + 1)] = yTc.T.reshape(BC, S, E)
    return y


def kernel(x, W_attn, b_attn, W_proj, b_proj):
    nc = _get_nc()
    maps = make_inputs(x, W_attn, b_attn, W_proj, b_proj)
    res = run_bass_kernel_spmd(nc, maps, list(range(N_CORES)))
    return assemble_output(res.results)



# revision 6
# speedup vs baseline: 16.4779x; 16.4779x over previous
"""Causal self-attention Trainium2 kernel.

B=1024, S=77, E=1024, H=16, D=64. Data-parallel over batch across 8 cores
(128 batches/core). bf16 on the PE with fp32 PSUM accumulation.

Scores are computed transposed (S^T[k,q] = K @ Q^T), so
exp(S^T) IS att^T — the AV matmul lhsT — with no per-pair PE transpose.
Causal masking is a zero-fill affine_select after exp (exp>0 everywhere, so
zeroing masked slots is exact). V is computed with an interleaved ones
column per head ([V_h | 1] at 65h..65h+65), so the AV matmul emits the
softmax normalizer z as column 64 of each head's output, already in
q-partition layout where a per-partition broadcast multiply normalizes.
The normalized O [77q, 1024] is PE-transposed per 128-feature chunk for
the projection.

Layout (per core, T = 128*77 = 9856 tokens):
  xT       [E=1024, T]     bf16 (host pre-transposed)
  Q^T,K^T  [2048 f, T]     bf16 = W_qk^T @ x^T  (f on partitions, chunks 128)
  V_aug    [77, 1040] x6   bf16 = x_b @ Wv_aug  (tokens on partitions,
                            head h at cols 65h..65h+64, ones at 65h+64)
  S^T      [77 k, 77 q]    head-pair row-packed matmuls (lhsT = K^T)
  att^T    exp on ACT -> bf16 sbuf, causal zero-fill on GpSimd (affine_select)
  O'       [77 q, 65*n]    = att^T.T @ V'_h  (z in col 64 of each slot)
  O        [77 q, 1024]    = O'[:, :64] * recip(z)  (per-partition bc)
  O^T      [128 d, 77*G]   PE transposes, 6 per psum bank (one per batch)
  y^T      [E, T] f32      = W_p^T @ O^T + b_p, host transposes back
"""

import sys

sys.path.insert(0, "/opt/trn_rl_repo")

import numpy as np
import ml_dtypes

import concourse.bass as bass
import concourse.mybir as mybir
import concourse.tile as tile
from concourse import bacc
from concourse.bass_utils import run_bass_kernel_spmd

F32 = mybir.dt.float32
BF16 = mybir.dt.bfloat16
AX = mybir.AxisListType
AF = mybir.ActivationFunctionType
ALU = mybir.AluOpType

N_CORES = 8
B, S, E = 1024, 77, 1024
H, D = 16, 64
BC = B // N_CORES          # batches per core = 128
T = BC * S                 # tokens per core = 9856
SCALE = 1.0 / float(np.sqrt(D))

# block structure: 21 blocks of 6 batches + 1 block of 2
BLOCKS = [(i * 6, 6) for i in range(21)] + [(126, 2)]

# AV psum banking: heads per bank (65 fp32 cols per head, <=7 per bank)
AV_BANKS = [(0, 7), (7, 7), (14, 2)]


def _emit_block(nc, tc, P, b0, G):
    Tb = G * S                       # tokens this block
    t0 = b0 * S
    xt = []
    for e in range(8):
        xtile = P["x"].tile([128, Tb], BF16, tag=f"xt{e}")
        nc.sync.dma_start(xtile[:], P["xT"][128 * e:128 * (e + 1), t0:t0 + Tb])
        xt.append(xtile)

    # ---- Q^T / K^T GEMM: 16 f-chunks of 128, contraction over 8 e-chunks
    qk = []
    for c in range(16):
        ps = P["gps"].tile([128, 512], F32, tag="g")
        for e in range(8):
            nc.tensor.matmul(
                ps[:, :Tb],
                P["wqk"][e][:, 128 * c:128 * (c + 1)],
                xt[e][:],
                start=(e == 0), stop=(e == 7),
            )
        o = P["qk"].tile([128, Tb], BF16, tag=f"qk{c}")
        # Identity(ps*scale + bias): SCALE folded into Q here (bias pre-scaled)
        nc.scalar.activation(
            o[:], ps[:, :Tb], AF.Identity,
            bias=P["bqk"][:, c:c + 1], scale=(SCALE if c < 8 else 1.0),
        )
        qk.append(o)

    # ---- V GEMM per batch: out [77 tokens, 1040 f] (ones cols via bv_aug)
    vb = []
    for g in range(G):
        v = P["v"].tile([77, 1040], BF16, tag=f"v{g}")
        for fc in range(4):
            ps = P["gps"].tile([128, 512], F32, tag="g")
            for e in range(8):
                nc.tensor.matmul(
                    ps[:77, :260],
                    xt[e][:, S * g:S * (g + 1)],
                    P["wv"][e][:, 260 * fc:260 * (fc + 1)],
                    start=(e == 0), stop=(e == 7),
                )
            nc.vector.tensor_add(
                v[:, 260 * fc:260 * (fc + 1)],
                ps[:77, :260],
                P["bv"][:77, 260 * fc:260 * (fc + 1)],
            )
        vb.append(v)

    # ---- scores S^T + exp + causal zero-fill.
    # HW constraint: matmuls within one PSUM bank must not alternate row
    # groups -> even heads (K^T rows 0:64) and odd heads (rows 64:128) go to
    # separate banks, emitted interleaved so PE overlaps LDWEIGHTS.
    def _exp_mask(sc, n):
        W = n * S
        eb = P["eb"].tile([77, 462], BF16, tag="eb", name="eb")
        nc.scalar.activation(eb[:, :W], sc[:, :W], AF.Exp)
        aT = P["attT"].tile([77, 462], BF16, tag="attT", name="attT")
        # keep k <= q within each pair: iota = q - k >= 0
        nc.gpsimd.affine_select(
            aT[:, :W].rearrange("p (n k) -> p n k", k=S),
            eb[:, :W].rearrange("p (n k) -> p n k", k=S),
            pattern=[[0, n], [1, S]],
            compare_op=ALU.is_ge,
            fill=0.0,
            base=0,
            channel_multiplier=-1,
        )
        return aT

    evens = [(g, h) for g in range(G) for h in range(0, H, 2)]
    odds = [(g, h) for g in range(G) for h in range(1, H, 2)]
    egroups = [evens[i:i + 6] for i in range(0, len(evens), 6)]
    ogroups = [odds[i:i + 6] for i in range(0, len(odds), 6)]
    attT = []
    pair_loc = {}
    for eg, og in zip(egroups, ogroups):
        scA_f = P["scps"].tile([128, 512], F32, tag="sc", name="sc")
        scB_f = P["scps"].tile([128, 512], F32, tag="sc", name="sc")
        scA, scB = scA_f[:77, :], scB_f[:77, :]
        for i in range(len(eg)):
            gA, hA = eg[i]
            gB, hB = og[i]
            cA, cB = hA // 2, hB // 2
            # S^T = K @ Q^T: lhsT = K^T chunk, rhs = Q^T chunk
            nc.tensor.matmul(
                scA[:, S * i:S * (i + 1)],
                qk[8 + cA][0:64, S * gA:S * (gA + 1)],
                qk[cA][0:64, S * gA:S * (gA + 1)],
                start=True, stop=True,
            )
            nc.tensor.matmul(
                scB[:, S * i:S * (i + 1)],
                qk[8 + cB][64:128, S * gB:S * (gB + 1)],
                qk[cB][64:128, S * gB:S * (gB + 1)],
                start=True, stop=True,
            )
        aT_A = _exp_mask(scA, len(eg))
        for i, pr in enumerate(eg):
            pair_loc[pr] = (len(attT), i)
        attT.append(aT_A)
        aT_B = _exp_mask(scB, len(og))
        for i, pr in enumerate(og):
            pair_loc[pr] = (len(attT), i)
        attT.append(aT_B)

    # ---- AV + normalize, per batch: O'[77q, 65] per head, z in col 64
    ob = []
    for g in range(G):
        o_b = P["ob"].tile([77, 1024], BF16, tag=f"ob{g}")
        for h0, nh in AV_BANKS:
            av_f = P["avps"].tile([128, 512], F32, tag="av", name="av")
            av = av_f[:77, :]
            for j in range(nh):
                h = h0 + j
                ti, si = pair_loc[(g, h)]
                nc.tensor.matmul(
                    av[:, 65 * j:65 * j + 65],
                    attT[ti][:, S * si:S * (si + 1)],
                    vb[g][:, 65 * h:65 * h + 65],
                    start=True, stop=True,
                )
            zr = P["z"].tile([77, 8], F32, tag="zr", name="zr")
            av3 = av[:, :65 * nh].rearrange("p (n c) -> p n c", c=65)
            nc.vector.reciprocal(zr[:, :nh].unsqueeze(2), av3[:, :, 64:65])
            zr_bc = zr[:, :nh].unsqueeze(2).broadcast_to([77, nh, 64])
            nc.vector.tensor_mul(
                o_b[:, 64 * h0:64 * (h0 + nh)].rearrange(
                    "p (n c) -> p n c", c=64),
                av3[:, :, 0:64],
                zr_bc,
            )
        ob.append(o_b)

    # ---- O^T: per d-chunk, transpose all G batches into one psum bank
    # bf16 PSUM writes need 4B alignment -> pad per-batch slots to 78 cols
    ot = []
    for dc in range(8):
        tp = P["tpps"].tile([128, 512], BF16, tag="tp", name="tp")
        for g in range(G):
            nc.tensor.transpose(
                tp[:, 78 * g:78 * g + S],
                ob[g][:, 128 * dc:128 * (dc + 1)],
                P["ident"][:],
            )
        o = P["ot"].tile([128, Tb], BF16, tag=f"ot{dc}")
        nc.scalar.activation(
            o[:].rearrange("p (g k) -> p g k", k=S),
            tp[:, :78 * G].rearrange("p (g k) -> p g k", k=78)[:, :, :S],
            AF.Copy,
        )
        ot.append(o)

    # ---- projection: y^T[e-chunk, t] = sum_dc Wp[dc]^T @ O^T[dc]
    for ec in range(8):
        ps = P["gps"].tile([128, 512], F32, tag="g")
        for dc in range(8):
            nc.tensor.matmul(
                ps[:, :Tb],
                P["wp"][dc][:, 128 * ec:128 * (ec + 1)],
                ot[dc][:],
                start=(dc == 0), stop=(dc == 7),
            )
        y = P["y"].tile([128, Tb], F32, tag="y")
        nc.scalar.activation(
            y[:], ps[:, :Tb], AF.Identity, bias=P["bp"][:, ec:ec + 1]
        )
        nc.sync.dma_start(P["yT"][128 * ec:128 * (ec + 1), t0:t0 + Tb], y[:])


def build(blocks=None, n_reps=1):
    if blocks is None:
        blocks = BLOCKS
    nc = bacc.Bacc(None)
    xT = nc.dram_tensor("xT", [E, T], BF16, kind="ExternalInput")
    wqk_d = nc.dram_tensor("wqk", [E, 2048], BF16, kind="ExternalInput")
    wv_d = nc.dram_tensor("wv", [E, 1040], BF16, kind="ExternalInput")
    wp_d = nc.dram_tensor("wp", [1024, 1024], BF16, kind="ExternalInput")
    bqk_d = nc.dram_tensor("bqk", [128, 16], F32, kind="ExternalInput")
    bv_d = nc.dram_tensor("bv", [128, 1040], F32, kind="ExternalInput")
    bp_d = nc.dram_tensor("bp", [128, 8], F32, kind="ExternalInput")
    id_d = nc.dram_tensor("ident", [77, 77], BF16, kind="ExternalInput")
    yT = nc.dram_tensor("yT", [E, T], F32, kind="ExternalOutput")

    with tile.TileContext(nc) as tc:
        with (
            tc.tile_pool(name="w", bufs=1) as wpool,
            tc.tile_pool(name="x", bufs=2) as xpool,
            tc.tile_pool(name="qk", bufs=1) as qkpool,
            tc.tile_pool(name="v", bufs=1) as vpool,
            tc.tile_pool(name="eb", bufs=4) as ebpool,
            tc.tile_pool(name="z", bufs=4) as zpool,
            tc.tile_pool(name="attT", bufs=18) as attTpool,
            tc.tile_pool(name="ob", bufs=2) as obpool,
            tc.tile_pool(name="ot", bufs=2) as otpool,
            tc.tile_pool(name="y", bufs=3) as ypool,
            tc.tile_pool(name="gps", bufs=2, space="PSUM") as gpspool,
            tc.tile_pool(name="scps", bufs=2, space="PSUM") as scpool,
            tc.tile_pool(name="avps", bufs=2, space="PSUM") as avpool,
            tc.tile_pool(name="tpps", bufs=2, space="PSUM") as tppool,
        ):
            P = {}
            P["wqk"] = []
            P["wv"] = []
            P["wp"] = []
            for e in range(8):
                w1 = wpool.tile([128, 2048], BF16, tag=f"wqk{e}", name=f"wqk{e}")
                nc.sync.dma_start(w1[:], wqk_d[128 * e:128 * (e + 1), :])
                P["wqk"].append(w1)
                w2 = wpool.tile([128, 1040], BF16, tag=f"wv{e}", name=f"wv{e}")
                nc.sync.dma_start(w2[:], wv_d[128 * e:128 * (e + 1), :])
                P["wv"].append(w2)
                w3 = wpool.tile([128, 1024], BF16, tag=f"wp{e}", name=f"wp{e}")
                nc.sync.dma_start(w3[:], wp_d[128 * e:128 * (e + 1), :])
                P["wp"].append(w3)
            P["bqk"] = wpool.tile([128, 16], F32, tag="bqk", name="bqk")
            nc.sync.dma_start(P["bqk"][:], bqk_d[:])
            P["bv"] = wpool.tile([128, 1040], F32, tag="bv", name="bv")
            nc.sync.dma_start(P["bv"][:], bv_d[:])
            P["bp"] = wpool.tile([128, 8], F32, tag="bp", name="bp")
            nc.sync.dma_start(P["bp"][:], bp_d[:])
            P["ident"] = wpool.tile([77, 77], BF16, tag="ident", name="ident")
            nc.sync.dma_start(P["ident"][:], id_d[:])
            P["xT"] = xT
            P["yT"] = yT
            P["x"] = xpool
            P["qk"] = qkpool
            P["v"] = vpool
            P["eb"] = ebpool
            P["z"] = zpool
            P["attT"] = attTpool
            P["ob"] = obpool
            P["ot"] = otpool
            P["y"] = ypool
            P["gps"] = gpspool
            P["scps"] = scpool
            P["avps"] = avpool
            P["tpps"] = tppool

            if n_reps == 1:
                for b0, G in blocks:
                    _emit_block(nc, tc, P, b0, G)
            else:
                with tc.For_i(0, n_reps, 1):
                    for b0, G in blocks:
                        _emit_block(nc, tc, P, b0, G)

    nc.finalize()
    return nc


_CACHE = {}


def _get_nc():
    if "nc" not in _CACHE:
        _CACHE["nc"] = build()
    return _CACHE["nc"]


def make_inputs(x, W_attn, b_attn, W_proj, b_proj):
    """Host-side prep: shard + transpose + cast. Returns per-core input maps."""
    x = np.asarray(x, dtype=np.float32)
    W_attn = np.asarray(W_attn, dtype=np.float32)
    b_attn = np.asarray(b_attn, dtype=np.float32)
    W_proj = np.asarray(W_proj, dtype=np.float32)
    b_proj = np.asarray(b_proj, dtype=np.float32)

    wqk = W_attn[:, :2048].astype(ml_dtypes.bfloat16)
    # V weights interleaved: head h at cols 65h..65h+64, zero col at 65h+64
    wv_aug = np.zeros((E, 1040), np.float32)
    wv3 = wv_aug.reshape(E, 16, 65)
    wv3[:, :, :64] = W_attn[:, 2048:].reshape(E, 16, 64)
    wv_aug = wv_aug.astype(ml_dtypes.bfloat16)
    wp = W_proj.astype(ml_dtypes.bfloat16)
    # bias chunks [128, 16]: col c = b_attn[128c:128c+128]; Q part pre-scaled
    bq = b_attn[:2048].copy()
    bq[:1024] *= SCALE
    bqk = np.stack([bq[128 * c:128 * (c + 1)] for c in range(16)], axis=1).astype(np.float32)
    bv_row = np.zeros((1040,), np.float32)
    bv3 = bv_row.reshape(16, 65)
    bv3[:, :64] = b_attn[2048:].reshape(16, 64)
    bv3[:, 64] = 1.0
    bv = np.broadcast_to(bv_row, (128, 1040)).copy().astype(np.float32)
    bp = np.stack([b_proj[128 * c:128 * (c + 1)] for c in range(8)], axis=1).astype(np.float32)
    ident = np.eye(77, dtype=np.float32).astype(ml_dtypes.bfloat16)

    maps = []
    for cid in range(N_CORES):
        xs = x[BC * cid:BC * (cid + 1)].reshape(T, E)
        xTc = np.ascontiguousarray(xs.T).astype(ml_dtypes.bfloat16)
        maps.append({
            "xT": xTc, "wqk": wqk, "wv": wv_aug, "wp": wp,
            "bqk": bqk, "bv": bv, "bp": bp, "ident": ident,
        })
    return maps


def assemble_output(results):
    y = np.empty((B, S, E), dtype=np.float32)
    for cid in range(N_CORES):
        yTc = results[cid]["yT"]  # [E, T]
        y[BC * cid:BC * (cid + 1)] = yTc.T.reshape(BC, S, E)
    return y


def kernel(x, W_attn, b_attn, W_proj, b_proj):
    nc = _get_nc()
    maps = make_inputs(x, W_attn, b_attn, W_proj, b_proj)
    res = run_bass_kernel_spmd(nc, maps, list(range(N_CORES)))
    return assemble_output(res.results)
